# revision 18
# baseline (speedup 1.0000x reference)
"""Trainium2 Bass kernel for nn_Guesser_2559800508528 (sparse_attention), v2.

Math (restructured from the reference):
  ghT[d,b]  = sum_k G[b,k,d] / K          (gsum: DVE grouped reduce on G^T)
  a1T[h,b]  = ghT @ W1q                   (PE bf16, transposed layout)
  h[h,b*t]  = relu(X^T(fp8) @ W1x(fp8) [DoubleRow] + a1 + b1)   -> h8 (fp8)
  e[1,rows] = w2 . h8                     (PE fp8 DoubleRow)
  u = exp(e); alpha = u / sum_t u         (strips)
  xhT[d,b]  = sum_t alpha[t] * X^T[d,t]   (DVE bcast-mult + grouped reduce)
  a2[b,j]   = xhT @ M1x' + c1'            (PE bf16; m2-sign-permuted, |m2|-folded)
  h2[row,j] = relu(G^T @ M1g' + a2-inject)   (natural layout, PE bf16)
  logits    = sum_pos relu - sum_neg relu + c2   (ACT/DVE accum_out, no matmul)

All device inputs are pre-transposed / pre-cast on the host (free): fp8 pairs
for DoubleRow, bf16 elsewhere. Output [128, 64] is reordered on host.

Sharding: pure data parallel over batch (512 batches per core x 8 cores).
"""

import numpy as np
import ml_dtypes

import concourse.bass as bass
import concourse.mybir as mybir
import concourse.tile as tile
from concourse import bacc
from concourse.bass_utils import run_bass_kernel_spmd

dt = mybir.dt
AF = mybir.ActivationFunctionType
ALU = mybir.AluOpType
F8 = ml_dtypes.float8_e4m3
BF = ml_dtypes.bfloat16

NCORES = 8
B_CORE = 512
T = 32
K = 16
D = 512
H = 1024
H2 = 2048
NX = B_CORE * T          # 16384 X rows per core
NG = B_CORE * K          # 8192 G rows per core
RT_X = NX // 512         # 32 row-tiles of 512 rows (16 batches)
RT_G = NG // 512         # 16 row-tiles of 512 rows (32 batches)

# engine split knobs
C_RELU_DVE = set()        # h slices evacuated on DVE (rest on ACT)
F_ACC_DVE = set()               # F 1024-blocks accumulated on DVE (rest on ACT)


def build_nc(P, reps=1):
    """P = sign boundary; reps>1 repeats the compute body (timing builds)."""
    nc = bacc.Bacc("TRN2", target_bir_lowering=False, debug=False)

    xt8_d = nc.declare_dram_parameter("XT8", [256, 2 * NX], dt.float8e4, isOutput=False).ap()
    xtb_d = nc.declare_dram_parameter("XTB", [128, 4 * NX], dt.bfloat16, isOutput=False).ap()
    gtb_d = nc.declare_dram_parameter("GTB", [128, 4 * NG], dt.bfloat16, isOutput=False).ap()
    w1x8_d = nc.declare_dram_parameter("W1X8", [256, 2048], dt.float8e4, isOutput=False).ap()
    w1qb_d = nc.declare_dram_parameter("W1QB", [512, 1024], dt.bfloat16, isOutput=False).ap()
    w28_d = nc.declare_dram_parameter("W28", [128, 128], dt.float8e4, isOutput=False).ap()
    m1xb_d = nc.declare_dram_parameter("M1XB", [512, 2048], dt.bfloat16, isOutput=False).ap()
    m1gb_d = nc.declare_dram_parameter("M1GB", [512, 2048], dt.bfloat16, isOutput=False).ap()
    c1r_d = nc.declare_dram_parameter("C1R", [1, 2048], dt.bfloat16, isOutput=False).ap()
    b1_d = nc.declare_dram_parameter("B1", [1024, 1], dt.float32, isOutput=False).ap()
    bindf_d = nc.declare_dram_parameter("BINDF", [128, 512], dt.bfloat16, isOutput=False).ap()
    c2_d = nc.declare_dram_parameter("C2", [1, 1], dt.float32, isOutput=False).ap()
    out_d = nc.declare_dram_parameter("out", [128, 64], dt.float32, isOutput=True).ap()

    with tile.TileContext(nc) as tc:
        _body(nc, tc, P, xt8_d, xtb_d, gtb_d, w1x8_d, w1qb_d, w28_d,
              m1xb_d, m1gb_d, c1r_d, b1_d, bindf_d, c2_d, out_d, reps)
    nc.compile()
    return nc


def _f_ranges(P):
    """Accum ranges (blk, a, b, col, kind) given sign boundary P.

    Columns of M1g'/M1x'/c1' are scaled by SIGNED m2, so positive-m2 columns
    contribute relu(pre') and negative-m2 columns contribute min(pre', 0);
    the logits strip is then a plain sum of all accumulator columns + c2.
    """
    rngs = []
    col = 0
    for blk in range(2):
        lo, hi = 1024 * blk, 1024 * blk + 1024
        pa, pb_ = lo, min(P, hi)
        if pb_ > pa:
            rngs.append((blk, pa - lo, pb_ - lo, col, "pos"))
            col += 1
        na, nb = max(P, lo), hi
        if nb > na:
            rngs.append((blk, na - lo, nb - lo, col, "neg"))
            col += 1
    npos = len([r for r in rngs if r[4] == "pos"])
    # reorder cols: pos ranges first, then neg
    rngs = sorted(rngs, key=lambda r: r[4] != "pos")
    rngs = [(blk, a, b, i, kind) for i, (blk, a, b, _, kind) in enumerate(rngs)]
    return rngs, col, npos


def _body(nc, tc, P, xt8_d, xtb_d, gtb_d, w1x8_d, w1qb_d, w28_d,
          m1xb_d, m1gb_d, c1r_d, b1_d, bindf_d, c2_d, out_d, reps=1):
    from contextlib import ExitStack
    ctx = ExitStack()
    f_ranges, n_acc, n_pos = _f_ranges(P)
    with ctx:
        persist = ctx.enter_context(tc.tile_pool(name="persist", bufs=1))
        # ---- persistent weights (direct DMA, pre-cast on host) ----
        w1x8 = [persist.tile([128, 2048], dt.float8e4, tag=f"w1x8_{p}", name=f"w1x8_{p}")
                for p in range(2)]
        for p in range(2):
            nc.sync.dma_start(w1x8[p][:], w1x8_d[128 * p:128 * (p + 1), :])
        w1qb = [persist.tile([128, 1024], dt.bfloat16, tag=f"w1qb_{d_}", name=f"w1qb_{d_}")
                for d_ in range(4)]
        m1xb = [persist.tile([128, 2048], dt.bfloat16, tag=f"m1xb_{d_}", name=f"m1xb_{d_}")
                for d_ in range(4)]
        m1gb = [persist.tile([128, 2048], dt.bfloat16, tag=f"m1gb_{d_}", name=f"m1gb_{d_}")
                for d_ in range(4)]
        for d_ in range(4):
            nc.sync.dma_start(w1qb[d_][:], w1qb_d[128 * d_:128 * (d_ + 1), :])
        w28 = persist.tile([128, 128], dt.float8e4, tag="w28", name="w28")
        nc.sync.dma_start(w28[:], w28_d[:, :])
        bindfh = [persist.tile([64, 512], dt.bfloat16, tag=f"bindf_{j}", name=f"bindf_{j}")
                  for j in range(2)]
        for j in range(2):
            nc.sync.dma_start(bindfh[j][:], bindf_d[64 * j:64 * (j + 1), :])
        b1s = [persist.tile([128, 1], dt.float32, tag=f"b1_{s}", name=f"b1_{s}")
               for s in range(8)]
        for s in range(8):
            nc.sync.dma_start(b1s[s][:], b1_d[128 * s:128 * (s + 1), :])
        c1r = persist.tile([1, 2048], dt.bfloat16, tag="c1r", name="c1r")
        nc.sync.dma_start(c1r[:], c1r_d[:, :])
        c2t = persist.tile([1, 1], dt.float32, tag="c2", name="c2")
        nc.sync.dma_start(c2t[:], c2_d[:, :])

        # broadcast constants across partitions (gpsimd)
        c1_128 = persist.tile([128, 2048], dt.bfloat16, tag="c1_128", name="c1_128")
        nc.gpsimd.partition_broadcast(c1_128[:], c1r[:])
        c2_128 = persist.tile([128, 1], dt.float32, tag="c2_128", name="c2_128")
        nc.gpsimd.partition_broadcast(c2_128[:], c2t[:])

        # ---- persistent activations ----
        ghT = persist.tile([128, 2048], dt.bfloat16, tag="ghT", name="ghT")
        xhT = persist.tile([128, 2048], dt.bfloat16, tag="xhT", name="xhT")
        a1T = [persist.tile([128, 512], dt.bfloat16, tag=f"a1T_{s}", name=f"a1T_{s}")
               for s in range(8)]
        a2n = [[persist.tile([64, 2048], dt.bfloat16, tag=f"a2n_{g}_{j}",
                             name=f"a2n_{g}_{j}") for j in range(2)]
               for g in range(4)]
        logT = persist.tile([128, 64], dt.float32, tag="logT", name="logT")

        # ---- working pools ----
        xq = ctx.enter_context(tc.tile_pool(name="xq", bufs=3))
        xb = ctx.enter_context(tc.tile_pool(name="xb", bufs=3))
        gp = ctx.enter_context(tc.tile_pool(name="gp", bufs=2))
        h8p = ctx.enter_context(tc.tile_pool(name="h8p", bufs=3))
        up = ctx.enter_context(tc.tile_pool(name="up", bufs=3))
        xup = ctx.enter_context(tc.tile_pool(name="xup", bufs=3))
        st = ctx.enter_context(tc.tile_pool(name="st", bufs=3))
        ps_c = ctx.enter_context(tc.tile_pool(name="ps_c", bufs=3, space="PSUM"))
        ps_f = ctx.enter_context(tc.tile_pool(name="ps_f", bufs=2, space="PSUM"))
        ps_e = ctx.enter_context(tc.tile_pool(name="ps_e", bufs=1, space="PSUM"))

        ghT_v = ghT[:].rearrange("p (d b) -> p d b", d=4)
        xhT_v = xhT[:].rearrange("p (d b) -> p d b", d=4)

        # Software-pipelined over 4 batch-groups of 128: prologue B/A for
        # group 0; then per group g: C(g), B/A(g+1) (fills PE while C(g)'s
        # softmax/xhat tail drains), E(g), F(g). The G tiles loaded for
        # gsum stay resident and are reused by F (G read once from HBM).
        gts_by_g = {}

        def emit_BA(g):
            gts = []
            for i in range(4):
                rt = 4 * g + i
                gt = gp.tile([128, 2048], dt.bfloat16, tag=f"gt_{i}",
                             name=f"gt_{i}")
                gts.append(gt)
                nc.sync.dma_start(gt[:], gtb_d[:, 2048 * rt:2048 * (rt + 1)])
                with nc.allow_low_precision("gsum bf16 out"):
                    nc.vector.reduce_sum(
                        ghT_v[:, :, 32 * rt:32 * (rt + 1)],
                        gt[:].rearrange("p (d b k) -> p d b k", d=4, k=K),
                        axis=mybir.AxisListType.X)
            gts_by_g[g] = gts
            for sp in range(4):
                ap_ = ps_c.tile([128, 512], dt.float32, tag="c", name="c")
                for half in range(2):
                    s = 2 * sp + half
                    sub = ap_[:, 256 * half:256 * half + 128]
                    for d_ in range(4):
                        nc.tensor.matmul(sub,
                                         w1qb[d_][:, 128 * s:128 * (s + 1)],
                                         ghT_v[:, d_, 128 * g:128 * (g + 1)],
                                         start=(d_ == 0), stop=(d_ == 3))
                    nc.scalar.activation(a1T[s][:, 128 * g:128 * (g + 1)],
                                         sub, AF.Copy)

        def emit_C_rt(rt):
            if True:
                x8 = [xq.tile([128, 1024], dt.float8e4, tag=f"x8_{p}",
                              name=f"x8_{p}") for p in range(2)]
                for p in range(2):
                    nc.scalar.dma_start(
                        x8[p][:], xt8_d[128 * p:128 * (p + 1),
                                        1024 * rt:1024 * (rt + 1)])
                xbt = xb.tile([128, 2048], dt.bfloat16, tag="xb", name="xb")
                nc.sync.dma_start(xbt[:], xtb_d[:, 2048 * rt:2048 * (rt + 1)])

                h8s = [h8p.tile([128, 1024], dt.float8e4, tag=f"h8_{sp}",
                                name=f"h8_{sp}") for sp in range(4)]
                for sp in range(4):
                    for half in range(2):
                        s = 2 * sp + half
                        hp = ps_c.tile([128, 512], dt.float32, tag="c", name="c")
                        sub = hp[:, :]
                        for p in range(2):
                            nc.tensor.matmul(
                                sub,
                                w1x8[p][:, 256 * s:256 * (s + 1)].rearrange(
                                    "p (k m) -> p k m", k=2),
                                x8[p][:].rearrange("p (k n) -> p k n", k=2),
                                start=(p == 0), stop=(p == 1),
                                perf_mode=mybir.MatmulPerfMode.DoubleRow)
                        h3 = sub.rearrange("p (b t) -> p b t", t=T)
                        a1b = a1T[s][:, 16 * rt:16 * (rt + 1)].unsqueeze(-1) \
                            .broadcast_to((128, 16, T))
                        nc.vector.tensor_tensor(h3, h3, a1b, ALU.add)
                        dst = h8s[sp][:, 512 * half:512 * half + 512]
                        if s in C_RELU_DVE:
                            nc.vector.tensor_scalar(dst, sub, b1s[s][:, 0:1],
                                                    0.0, ALU.add, op1=ALU.max)
                        else:
                            nc.scalar.activation(dst, sub, AF.Relu,
                                                 bias=b1s[s][:])

                ep = ps_e.tile([1, 512], dt.float32, tag="e", name="e")
                for sp in range(4):
                    nc.tensor.matmul(
                        ep[:], w28[:, 32 * sp:32 * sp + 32].rearrange(
                            "p (k m) -> p k m", k=2)[:, :, 0:1],
                        h8s[sp][:].rearrange("p (k n) -> p k n", k=2),
                        start=(sp == 0), stop=(sp == 3),
                        perf_mode=mybir.MatmulPerfMode.DoubleRow)

                u_row = st.tile([1, 512], dt.float32, tag="u_row", name="u_row")
                nc.scalar.activation(u_row[:], ep[:], AF.Exp)
                srow = st.tile([1, 16], dt.float32, tag="srow", name="srow")
                nc.vector.reduce_sum(srow[:],
                                     u_row[:].rearrange("p (b t) -> p b t", t=T),
                                     axis=mybir.AxisListType.X)
                rs = st.tile([1, 16], dt.float32, tag="rs", name="rs")
                nc.vector.reciprocal(rs[:], srow[:])
                un = st.tile([1, 512], dt.bfloat16, tag="un", name="un")
                nc.vector.tensor_tensor(
                    un[:].rearrange("p (b t) -> p b t", t=T),
                    u_row[:].rearrange("p (b t) -> p b t", t=T),
                    rs[:].unsqueeze(-1).broadcast_to((1, 16, T)), ALU.mult)
                u128 = up.tile([128, 512], dt.bfloat16, tag="u128", name="u128")
                nc.gpsimd.partition_broadcast(u128[:], un[:])
                xu = xup.tile([128, 2048], dt.bfloat16, tag="xu", name="xu")
                for dh in range(2):
                    nc.gpsimd.tensor_tensor(
                        xu[:, 1024 * dh:1024 * (dh + 1)].rearrange(
                            "p (d n) -> p d n", d=2),
                        xbt[:, 1024 * dh:1024 * (dh + 1)].rearrange(
                            "p (d n) -> p d n", d=2),
                        u128[:].unsqueeze(1).broadcast_to((128, 2, 512)),
                        ALU.mult)
                    with nc.allow_low_precision("xhat bf16 out"):
                        nc.vector.reduce_sum(
                            xhT_v[:, 2 * dh:2 * dh + 2, 16 * rt:16 * (rt + 1)],
                            xu[:, 1024 * dh:1024 * (dh + 1)].rearrange(
                                "p (d b t) -> p d b t", d=2, t=T),
                            axis=mybir.AxisListType.X)

        def emit_E(g):
            eqs = [ps_f.tile([128, 1024], dt.float32, tag="f", name="f")
                   for _ in range(2)]
            esubs = [eqs[q // 2][:, 512 * (q % 2):512 * (q % 2) + 512]
                     for q in range(4)]
            for d_ in range(4):
                stat = xhT_v[:, d_, 128 * g:128 * (g + 1)]
                for q in range(4):
                    nc.tensor.matmul(esubs[q], stat,
                                     m1xb[d_][:, 512 * q:512 * (q + 1)],
                                     start=(d_ == 0), stop=(d_ == 3))
            for q in range(4):
                for j in range(2):
                    nc.vector.tensor_tensor(
                        a2n[g][j][:, 512 * q:512 * (q + 1)],
                        esubs[q][64 * j:64 * (j + 1), :],
                        c1_128[64 * j:64 * (j + 1), 512 * q:512 * (q + 1)],
                        ALU.add)

        def emit_F_rt(rt):
            if True:
                g, i = rt // 4, rt % 4
                k_ = rt % 4
                gt = gts_by_g[g][i]
                for c in range(4):
                    acc = st.tile([128, 4], dt.float32, tag="acc", name="acc")
                    pqs = [ps_f.tile([128, 1024], dt.float32, tag="f", name="f")
                           for _ in range(2)]
                    subs = [pqs[q // 2][:, 512 * (q % 2):512 * (q % 2) + 512]
                            for q in range(4)]
                    # d outer / q inner: the gt stationary chunk is reused by
                    # 4 consecutive matmuls (weight reload is the HW cost)
                    for d_ in range(4):
                        stat = gt[:, 512 * d_ + 128 * c:512 * d_ + 128 * (c + 1)]
                        for q in range(4):
                            nc.tensor.matmul(
                                subs[q], stat,
                                m1gb[d_][:, 512 * q:512 * (q + 1)],
                                start=(d_ == 0), stop=False)
                    jh, kh = k_ // 2, k_ % 2
                    istat = bindfh[jh][32 * kh:32 * (kh + 1),
                                       128 * c:128 * (c + 1)]
                    for q in range(4):
                        nc.tensor.matmul(
                            subs[q], istat,
                            a2n[g][jh][32 * kh:32 * (kh + 1),
                                       512 * q:512 * (q + 1)],
                            start=False, stop=True)
                    for (blk, a, b, col, kind) in f_ranges:
                        pq = pqs[blk]
                        if kind == "pos" and blk == 1:
                            dum = st.tile([128, 1024], dt.float8e4, tag="dumV",
                                          name="dumV")
                            nc.vector.tensor_scalar(
                                dum[:, 0:b - a], pq[:, a:b], 0.0, None,
                                ALU.max, op1=ALU.add,
                                accum_out=acc[:, col:col + 1])
                        else:
                            # neg ranges: Relu(-x) accumulates -contribution
                            dum = st.tile([128, 1024], dt.float8e4, tag="dumA",
                                          name="dumA")
                            scale = 1.0 if kind == "pos" else -1.0
                            nc.scalar.activation(
                                dum[:, 0:b - a], pq[:, a:b], AF.Relu,
                                scale=scale, accum_out=acc[:, col:col + 1])
                    # logits strip: sum(pos cols) - sum(neg cols) + c2
                    lcol = logT[:, 4 * rt + c:4 * rt + c + 1]
                    tneg = st.tile([128, 1], dt.float32, tag="tneg", name="tneg")
                    nc.vector.reduce_sum(lcol, acc[:, 0:n_pos].unsqueeze(1),
                                         axis=mybir.AxisListType.X)
                    if n_acc > n_pos:
                        nc.vector.reduce_sum(tneg[:],
                                             acc[:, n_pos:n_acc].unsqueeze(1),
                                             axis=mybir.AxisListType.X)
                        nc.vector.tensor_tensor(lcol, lcol, tneg[:], ALU.subtract)
                    nc.vector.tensor_tensor(lcol, lcol, c2_128[:], ALU.add)

        first = True
        for _rep in range(reps):
            emit_BA(0)
            if first:
                # deferred E/F weight loads: not needed until E(0)/F(0), keep
                # the DMA queues free for the first G/X streams
                for d_ in range(4):
                    nc.scalar.dma_start(m1xb[d_][:],
                                        m1xb_d[128 * d_:128 * (d_ + 1), :])
                    nc.sync.dma_start(m1gb[d_][:],
                                      m1gb_d[128 * d_:128 * (d_ + 1), :])
                first = False
            for g in range(4):
                for i in range(8):
                    emit_C_rt(8 * g + i)
                    if g > 0 and i % 2 == 1:
                        emit_F_rt(4 * (g - 1) + i // 2)
                if g + 1 < 4:
                    emit_BA(g + 1)
                emit_E(g)
            for i in range(4):
                emit_F_rt(12 + i)

            nc.sync.dma_start(out_d[:, :], logT[:])


_NC_CACHE = {}


def _prep_shared(W1, b1, w2, M1, c1, m2, c2):
    W1q = (W1[:D] / K).astype(np.float32)
    W1x = W1[D:]
    pos = m2 >= 0
    sigma = np.concatenate([np.nonzero(pos)[0], np.nonzero(~pos)[0]])
    P = int(pos.sum())
    sm2 = m2  # signed scaling; neg columns use min(x,0) accumulation

    w1x8 = np.ascontiguousarray(
        W1x.reshape(2, 2, 128, 8, 128).transpose(0, 2, 3, 1, 4)
        .reshape(256, 2048)).astype(F8)
    w1qb = np.ascontiguousarray(W1q).astype(BF)
    w28 = np.zeros((128, 128), np.float32)
    for sp in range(4):
        w28[:, 32 * sp] = w2[128 * 2 * sp:128 * (2 * sp + 1)]
        w28[:, 32 * sp + 16] = w2[128 * (2 * sp + 1):128 * (2 * sp + 2)]
    w28 = w28.astype(F8)
    m1xb = np.ascontiguousarray((M1[:D][:, sigma] * sm2[sigma])).astype(BF)
    m1gb = np.ascontiguousarray((M1[D:][:, sigma] * sm2[sigma])).astype(BF)
    c1r = np.ascontiguousarray((c1[sigma] * sm2[sigma])[None, :]).astype(BF)
    b1h = np.ascontiguousarray(b1[:, None]).astype(np.float32)

    bindf = np.zeros((128, 512), np.float32)
    for row in range(128):
        b = row % 32
        bindf[row, 16 * b:16 * (b + 1)] = 1.0
    bindf = bindf.astype(BF)
    c2h = np.asarray(c2, np.float32).reshape(1, 1)
    shared = dict(W1X8=w1x8, W1QB=w1qb, W28=w28, M1XB=m1xb, M1GB=m1gb,
                  C1R=c1r, B1=b1h, BINDF=bindf, C2=c2h)
    return shared, P


def _prep_core(Xc, Gc):
    # Xc [NX, D] fp32; Gc [NG, D] fp32
    xt8 = np.ascontiguousarray(
        Xc.reshape(RT_X, 512, 2, 2, 128).transpose(2, 4, 0, 3, 1)
        .reshape(256, 2 * NX)).astype(F8)
    xtb = np.ascontiguousarray(
        Xc.reshape(RT_X, 512, 4, 128).transpose(3, 0, 2, 1)
        .reshape(128, 4 * NX)).astype(BF)
    gtb = np.ascontiguousarray(
        Gc.reshape(RT_G, 512, 4, 128).transpose(3, 0, 2, 1)
        .reshape(128, 4 * NG)).astype(BF)
    return dict(XT8=xt8, XTB=xtb, GTB=gtb)


def kernel(**inputs):
    X = np.asarray(inputs["X"], dtype=np.float32)
    G = np.asarray(inputs["G"], dtype=np.float32)
    W1 = np.asarray(inputs["W1"], dtype=np.float32)
    b1 = np.asarray(inputs["b1"], dtype=np.float32)
    w2 = np.asarray(inputs["w2"], dtype=np.float32)
    M1 = np.asarray(inputs["M1"], dtype=np.float32)
    c1 = np.asarray(inputs["c1"], dtype=np.float32)
    m2 = np.asarray(inputs["m2"], dtype=np.float32)
    c2 = np.asarray(inputs["c2"], dtype=np.float32)

    shared, P = _prep_shared(W1, b1, w2, M1, c1, m2, c2)
    if _NC_CACHE.get("P") != P:
        _NC_CACHE["nc"] = build_nc(P)
        _NC_CACHE["P"] = P
    nc = _NC_CACHE["nc"]

    in_maps = []
    for c in range(NCORES):
        m = dict(shared)
        m.update(_prep_core(
            X[c * B_CORE:(c + 1) * B_CORE].reshape(NX, D),
            G[c * B_CORE:(c + 1) * B_CORE].reshape(NG, D)))
        in_maps.append(m)

    _NC_CACHE["in_maps"] = in_maps
    res = run_bass_kernel_spmd(nc, in_maps, list(range(NCORES)))
    outs = []
    for c in range(NCORES):
        r = np.asarray(res.results[c]["out"], np.float32)   # [128, 64]
        outs.append(r.T.reshape(B_CORE, K))                 # rt-major -> rows
    return np.concatenate(outs, axis=0)
